# revision 11
# baseline (speedup 1.0000x reference)
"""Cost-volume block kernel for Trainium2 (8 NeuronCores, batch-sharded).

Computes, for c1/warp of shape [B, H, W, C] (B=8, H=192, W=640, C=32):
    cost[d] = mean_c( c1[..., c] * warp_shifted_by(d-2)[..., c] )   d in 0..4
    out     = concat([c1, cost_0..cost_4], axis=-1)                 # [B,H,W,37]

Strategy (v4 — multi-engine split, bf16, paired groups):
  - one batch per NeuronCore (8 cores), SPMD via run_bass_kernel_spmd.
  - host-side: inputs cast to bf16 and transposed to a channels-on-partition
    layout: partition p = r*32 + c (r = row-within-4-row-subgroup, c = chan),
    free dim = (sub-block s in 0..1, subgroup g in 0..3, pixel w).  Each
    "pair" P covers 32 image rows = 2 sub-blocks x 4 subgroups x 4 rows;
    6 pairs per core.
  - DVE does the only elementwise work: 5 bf16 tensor_tensor multiplies per
    pair (one per disparity offset, free dim 5120), running in 2x_1P packed
    mode.  Odd offsets read a 1-pixel-shifted copy of the warp tile (made by
    the otherwise-idle ScalarE; GpSimd would serialize with DVE via the
    shared SBUF port pair) so every operand stays 4B-aligned.
  - TensorE reduces channels: matmul with a sparse [128, 20] stationary
    (value 1/32, folding the channel mean) contracts the partition dim;
    stationary d routes row r's channel-sum to psum partition r*5 + d, and
    tile_position=(0, 32*g) places subgroup g's output at partition offset
    32*g.  5 matmuls per psum region accumulate the 5 offsets; each
    sub-block s has its own [128, 2, 512] psum tile (2 banks).
  - ScalarE evacuates PSUM -> SBUF (bf16); all DMA on HWDGE (sync for
    loads, scalar for the store).
  - host gathers [6, 2, 128, 640] bf16 per core back to [H, W, 5] f32 and
    concats the c1 passthrough channels (bit-exact, host-side).
"""

import sys

if "/opt/trn_rl_repo" not in sys.path:
    sys.path.insert(0, "/opt/trn_rl_repo")

import numpy as np
from ml_dtypes import bfloat16

# Problem constants (hardcoded per harness contract).
B, H, W, C = 8, 192, 640, 32
SR = 2                  # search range
NOFF = 2 * SR + 1       # 5 disparity offsets
OUTC = C + NOFF         # 37 output channels

NP = 6                  # pairs per core (32 rows each)
NS = 2                  # sub-blocks per pair
NSUB = 4                # subgroups (g) per sub-block
NR = 4                  # rows (r) per subgroup; partition p = r*32 + c
WB = W + 2 * SR + 4     # 648: padded halo width per subgroup block
NBLK = NS * NSUB        # 8 pixel blocks per pair
FW = NBLK * W           # 5120 free elems per c1 / product tile
FWP = NBLK * WB         # 5184 free elems per warp tile
M = NR * NOFF           # 20 psum partitions per subgroup: m = r*5 + d
NH = 2                  # psum bank halves (matmul N = 320)
NCHUNK = W // NH        # 320
DORD = [0, 2, 4, 1, 3]  # even offsets first: they don't wait on the wo copy

_BUILT = None


def _build():
    """Build + schedule the per-core Bass program (shapes are per-core)."""
    global _BUILT
    if _BUILT is not None:
        return _BUILT

    import concourse.bacc as bacc
    import concourse.mybir as mybir
    import concourse.tile as tile

    f32 = mybir.dt.float32
    bf16 = mybir.dt.bfloat16
    nc = bacc.Bacc("TRN2", target_bir_lowering=False, debug=False)
    c1T = nc.dram_tensor("c1t", [NP, 128, FW], bf16, kind="ExternalInput").ap()
    wpT = nc.dram_tensor("wpt", [NP, 128, FWP], bf16, kind="ExternalInput").ap()
    sON = nc.dram_tensor("sones", [128, NOFF * M], bf16,
                         kind="ExternalInput").ap()
    out = nc.dram_tensor("out", [NP, NS, 128, W], bf16,
                         kind="ExternalOutput").ap()

    with tile.TileContext(nc) as tc:
        with tc.tile_pool(name="const", bufs=1) as cons, \
             tc.tile_pool(name="ins", bufs=2) as ins, \
             tc.tile_pool(name="prod", bufs=2) as pr, \
             tc.tile_pool(name="psum", bufs=2, space="PSUM") as pp, \
             tc.tile_pool(name="outs", bufs=2) as outs:
            s_t = cons.tile([128, NOFF * M], bf16)
            for P in range(NP):
                # pair 0 is chunked so the first multiply starts as soon as
                # the first half of the load lands instead of the whole
                # 2.6 MB pair; steady-state pairs are prefetched in time
                if P == 0:          # ramp: start multiplying ASAP
                    chunks = [(0, 1), (1, 2), (2, 4), (4, 8)]
                elif P == NP - 1:   # drain: halve the final matmul tail
                    chunks = [(0, 4), (4, 8)]
                else:
                    chunks = [(0, NBLK)]
                c1_t = ins.tile([128, FW], bf16, tag="c1")
                we_t = ins.tile([128, FWP], bf16, tag="we")
                wo_t = ins.tile([128, FWP], bf16, tag="wo")
                for (b0, b1) in chunks:
                    nc.sync.dma_start(out=c1_t[:, b0 * W:b1 * W],
                                      in_=c1T[P][:, b0 * W:b1 * W])
                    nc.sync.dma_start(out=we_t[:, b0 * WB:b1 * WB],
                                      in_=wpT[P][:, b0 * WB:b1 * WB])
                if P == 0:
                    # needed first by the d=0 matmuls, not the first TT
                    nc.sync.dma_start(out=s_t, in_=sON)
                for (b0, b1) in chunks:
                    # odd-alignment copy: wo[j] = we[j+1], keeps odd-d
                    # operands 4B-aligned so the DVE multiply stays in 2x
                    # packed mode.  ScalarE has its own dedicated SBUF port;
                    # GpSimd would serialize with DVE via the shared pair.
                    nc.scalar.copy(out=wo_t[:, b0 * WB:b1 * WB - 1],
                                   in_=we_t[:, b0 * WB + 1:b1 * WB])
                ps = [pp.tile([128, NH, 512], f32, tag=f"ps{s}",
                              name=f"ps{s}")
                      for s in range(NS)]
                c1_3 = c1_t[:].rearrange("p (b w) -> p b w", b=NBLK)
                for di, d in enumerate(DORD):
                    src, off = (we_t, d) if d % 2 == 0 else (wo_t, d - 1)
                    p_t = pr.tile([128, FW], bf16, tag=f"p{d}")
                    w_3 = src[:].rearrange(
                        "p (b j) -> p b j", b=NBLK)[:, :, off:off + W]
                    for (b0, b1) in chunks:
                        nc.vector.tensor_mul(
                            p_t[:].rearrange(
                                "p (b w) -> p b w", b=NBLK)[:, b0:b1],
                            c1_3[:, b0:b1], w_3[:, b0:b1])
                    lhsT = s_t[:, d * M:(d + 1) * M]
                    for s in range(NS):
                        for g in range(NSUB):
                            base = (s * NSUB + g) * W
                            for h in range(NH):
                                nc.tensor.matmul(
                                    ps[s][32 * g:32 * g + M, h, 0:NCHUNK],
                                    lhsT,
                                    p_t[:, base + h * NCHUNK:
                                        base + (h + 1) * NCHUNK],
                                    start=(di == 0),
                                    stop=(di == NOFF - 1),
                                    tile_position=(0, 32 * g),
                                )
                for s in range(NS):
                    o_t = outs.tile([128, W], bf16, tag=f"o{s}")
                    nc.scalar.copy(
                        out=o_t[:].rearrange("p (a b) -> p a b", a=NH),
                        in_=ps[s][:, :, 0:NCHUNK])
                    # final stores ride the otherwise-idle sync ring so the
                    # drain isn't serialized behind the last psum copies
                    eng = nc.sync if P == NP - 1 else nc.scalar
                    eng.dma_start(out=out[P, s], in_=o_t[:])

    nc.compile()
    _BUILT = nc
    return _BUILT


def _prep_c1(c1):
    """[B, H, W, C] f32 -> [B, NP, 128, FW] bf16, partition p = r*32+c."""
    t = c1.reshape(B, NP, NS, NSUB, NR, W, C)       # b P s g r w c
    t = t.transpose(0, 1, 4, 6, 2, 3, 5)            # b P r c s g w
    return np.ascontiguousarray(t.reshape(B, NP, 128, FW)).astype(bfloat16)


def _prep_warph(warp):
    """[B, H, W, C] f32 -> haloed transposed [B, NP, 128, FWP] bf16."""
    wp = np.zeros((B, H, WB, C), dtype=np.float32)
    wp[:, :, SR:SR + W] = warp
    t = wp.reshape(B, NP, NS, NSUB, NR, WB, C)      # b P s g r j c
    t = t.transpose(0, 1, 4, 6, 2, 3, 5)            # b P r c s g j
    return np.ascontiguousarray(t.reshape(B, NP, 128, FWP)).astype(bfloat16)


def _make_sones():
    """[128, 5*20] bf16 stationaries; S_d[(r,c), m] = 1/32 iff m == r*5+d."""
    S = np.zeros((128, NOFF * M), dtype=np.float32)
    for d in range(NOFF):
        for r in range(NR):
            S[r * C:(r + 1) * C, d * M + r * NOFF + d] = 1.0 / C
    return S.astype(bfloat16)


def _run(c1_full, warph_full, trace=False, **kw):
    from concourse.bass_utils import run_bass_kernel_spmd

    nc = _build()
    c1t = _prep_c1(c1_full)
    sones = _make_sones()
    in_maps = [{"c1t": c1t[i], "wpt": warph_full[i], "sones": sones}
               for i in range(B)]
    return run_bass_kernel_spmd(nc, in_maps, list(range(B)), trace=trace, **kw)


def kernel(c1, warp, search_range):
    assert int(search_range) == SR, f"kernel hardcodes search_range={SR}"
    c1 = np.ascontiguousarray(np.asarray(c1, dtype=np.float32))
    warp = np.ascontiguousarray(np.asarray(warp, dtype=np.float32))
    assert c1.shape == (B, H, W, C) and warp.shape == (B, H, W, C)
    warph = _prep_warph(warp)
    r = _run(c1, warph, trace=False)
    out = np.empty((B, H, W, OUTC), dtype=np.float32)
    out[..., :C] = c1
    for i in range(B):
        cost = np.asarray(r.results[i]["out"]).astype(np.float32)
        # [NP, s, p=(g, q<=31), w]; valid q<20 encode (r, d) = (q//5, q%5)
        cost = cost.reshape(NP, NS, NSUB, 32, W)[:, :, :, :M, :]
        cost = cost.reshape(NP, NS, NSUB, NR, NOFF, W)
        cost = cost.transpose(0, 1, 2, 3, 5, 4)     # P s g r w d
        out[i, ..., C:] = cost.reshape(H, W, NOFF)
    return out


# revision 12
# speedup vs baseline: 1.0171x; 1.0171x over previous
"""Cost-volume block kernel for Trainium2 (8 NeuronCores, batch-sharded).

Computes, for c1/warp of shape [B, H, W, C] (B=8, H=192, W=640, C=32):
    cost[d] = mean_c( c1[..., c] * warp_shifted_by(d-2)[..., c] )   d in 0..4
    out     = concat([c1, cost_0..cost_4], axis=-1)                 # [B,H,W,37]

Strategy (v4 — multi-engine split, bf16, paired groups):
  - one batch per NeuronCore (8 cores), SPMD via run_bass_kernel_spmd.
  - host-side: inputs cast to bf16 and transposed to a channels-on-partition
    layout: partition p = r*32 + c (r = row-within-4-row-subgroup, c = chan),
    free dim = (sub-block s in 0..1, subgroup g in 0..3, pixel w).  Each
    "pair" P covers 32 image rows = 2 sub-blocks x 4 subgroups x 4 rows;
    6 pairs per core.
  - DVE does the only elementwise work: 5 bf16 tensor_tensor multiplies per
    pair (one per disparity offset, free dim 5120), running in 2x_1P packed
    mode.  Odd offsets read a 1-pixel-shifted copy of the warp tile (made by
    the otherwise-idle ScalarE; GpSimd would serialize with DVE via the
    shared SBUF port pair) so every operand stays 4B-aligned.
  - TensorE reduces channels: matmul with a sparse [128, 20] stationary
    (value 1/32, folding the channel mean) contracts the partition dim;
    stationary d routes row r's channel-sum to psum partition r*5 + d, and
    tile_position=(0, 32*g) places subgroup g's output at partition offset
    32*g.  5 matmuls per psum region accumulate the 5 offsets; each
    sub-block s has its own [128, 2, 512] psum tile (2 banks).
  - ScalarE evacuates PSUM -> SBUF (bf16); all DMA on HWDGE (sync for
    loads, scalar for the store).
  - host gathers [6, 2, 128, 640] bf16 per core back to [H, W, 5] f32 and
    concats the c1 passthrough channels (bit-exact, host-side).
"""

import sys

if "/opt/trn_rl_repo" not in sys.path:
    sys.path.insert(0, "/opt/trn_rl_repo")

import numpy as np
from ml_dtypes import bfloat16

# Problem constants (hardcoded per harness contract).
B, H, W, C = 8, 192, 640, 32
SR = 2                  # search range
NOFF = 2 * SR + 1       # 5 disparity offsets
OUTC = C + NOFF         # 37 output channels

NP = 6                  # pairs per core (32 rows each)
NS = 2                  # sub-blocks per pair
NSUB = 4                # subgroups (g) per sub-block
NR = 4                  # rows (r) per subgroup; partition p = r*32 + c
WB = W + 2 * SR + 4     # 648: padded halo width per subgroup block
NBLK = NS * NSUB        # 8 pixel blocks per pair
FW = NBLK * W           # 5120 free elems per c1 / product tile
FWP = NBLK * WB         # 5184 free elems per warp tile
M = NR * NOFF           # 20 psum partitions per subgroup: m = r*5 + d
NH = 2                  # psum bank halves (matmul N = 320)
NCHUNK = W // NH        # 320
DORD = [0, 2, 4, 1, 3]  # even offsets first: they don't wait on the wo copy

_BUILT = None


def _build():
    """Build + schedule the per-core Bass program (shapes are per-core)."""
    global _BUILT
    if _BUILT is not None:
        return _BUILT

    import concourse.bacc as bacc
    import concourse.mybir as mybir
    import concourse.tile as tile

    f32 = mybir.dt.float32
    bf16 = mybir.dt.bfloat16
    nc = bacc.Bacc("TRN2", target_bir_lowering=False, debug=False)
    c1T = nc.dram_tensor("c1t", [NP, 128, FW], bf16, kind="ExternalInput").ap()
    wpT = nc.dram_tensor("wpt", [NP, 128, FWP], bf16, kind="ExternalInput").ap()
    sON = nc.dram_tensor("sones", [128, NOFF * M], bf16,
                         kind="ExternalInput").ap()
    out = nc.dram_tensor("out", [NP, NS, 128, W], bf16,
                         kind="ExternalOutput").ap()

    with tile.TileContext(nc) as tc:
        with tc.tile_pool(name="const", bufs=1) as cons, \
             tc.tile_pool(name="ins", bufs=2) as ins, \
             tc.tile_pool(name="prod", bufs=2) as pr, \
             tc.tile_pool(name="psum", bufs=2, space="PSUM") as pp, \
             tc.tile_pool(name="outs", bufs=2) as outs:
            s_t = cons.tile([128, NOFF * M], bf16)
            for P in range(NP):
                # pair 0 is chunked so the first multiply starts as soon as
                # the first half of the load lands instead of the whole
                # 2.6 MB pair; steady-state pairs are prefetched in time
                if P == 0:          # ramp: start multiplying ASAP
                    chunks = [(0, 2), (2, 4), (4, 8)]
                elif P == NP - 1:   # drain: halve the final matmul tail
                    chunks = [(0, 4), (4, 8)]
                else:
                    chunks = [(0, NBLK)]
                c1_t = ins.tile([128, FW], bf16, tag="c1")
                we_t = ins.tile([128, FWP], bf16, tag="we")
                wo_t = ins.tile([128, FWP], bf16, tag="wo")
                for (b0, b1) in chunks:
                    nc.sync.dma_start(out=c1_t[:, b0 * W:b1 * W],
                                      in_=c1T[P][:, b0 * W:b1 * W])
                    nc.sync.dma_start(out=we_t[:, b0 * WB:b1 * WB],
                                      in_=wpT[P][:, b0 * WB:b1 * WB])
                if P == 0:
                    # needed first by the d=0 matmuls, not the first TT
                    nc.sync.dma_start(out=s_t, in_=sON)
                for (b0, b1) in chunks:
                    # odd-alignment copy: wo[j] = we[j+1], keeps odd-d
                    # operands 4B-aligned so the DVE multiply stays in 2x
                    # packed mode.  ScalarE has its own dedicated SBUF port;
                    # GpSimd would serialize with DVE via the shared pair.
                    nc.scalar.copy(out=wo_t[:, b0 * WB:b1 * WB - 1],
                                   in_=we_t[:, b0 * WB + 1:b1 * WB])
                ps = [pp.tile([128, NH, 512], f32, tag=f"ps{s}",
                              name=f"ps{s}")
                      for s in range(NS)]
                c1_3 = c1_t[:].rearrange("p (b w) -> p b w", b=NBLK)
                for di, d in enumerate(DORD):
                    src, off = (we_t, d) if d % 2 == 0 else (wo_t, d - 1)
                    p_t = pr.tile([128, FW], bf16, tag=f"p{d}")
                    w_3 = src[:].rearrange(
                        "p (b j) -> p b j", b=NBLK)[:, :, off:off + W]
                    for (b0, b1) in chunks:
                        nc.vector.tensor_mul(
                            p_t[:].rearrange(
                                "p (b w) -> p b w", b=NBLK)[:, b0:b1],
                            c1_3[:, b0:b1], w_3[:, b0:b1])
                    lhsT = s_t[:, d * M:(d + 1) * M]
                    for s in range(NS):
                        for g in range(NSUB):
                            base = (s * NSUB + g) * W
                            for h in range(NH):
                                nc.tensor.matmul(
                                    ps[s][32 * g:32 * g + M, h, 0:NCHUNK],
                                    lhsT,
                                    p_t[:, base + h * NCHUNK:
                                        base + (h + 1) * NCHUNK],
                                    start=(di == 0),
                                    stop=(di == NOFF - 1),
                                    tile_position=(0, 32 * g),
                                )
                for s in range(NS):
                    o_t = outs.tile([128, W], bf16, tag=f"o{s}")
                    nc.scalar.copy(
                        out=o_t[:].rearrange("p (a b) -> p a b", a=NH),
                        in_=ps[s][:, :, 0:NCHUNK])
                    # final stores ride the otherwise-idle sync ring so the
                    # drain isn't serialized behind the last psum copies
                    eng = nc.sync if P == NP - 1 else nc.scalar
                    eng.dma_start(out=out[P, s], in_=o_t[:])

    nc.compile()
    _BUILT = nc
    return _BUILT


def _prep_c1(c1):
    """[B, H, W, C] f32 -> [B, NP, 128, FW] bf16, partition p = r*32+c."""
    t = c1.reshape(B, NP, NS, NSUB, NR, W, C)       # b P s g r w c
    t = t.transpose(0, 1, 4, 6, 2, 3, 5)            # b P r c s g w
    return np.ascontiguousarray(t.reshape(B, NP, 128, FW)).astype(bfloat16)


def _prep_warph(warp):
    """[B, H, W, C] f32 -> haloed transposed [B, NP, 128, FWP] bf16."""
    wp = np.zeros((B, H, WB, C), dtype=np.float32)
    wp[:, :, SR:SR + W] = warp
    t = wp.reshape(B, NP, NS, NSUB, NR, WB, C)      # b P s g r j c
    t = t.transpose(0, 1, 4, 6, 2, 3, 5)            # b P r c s g j
    return np.ascontiguousarray(t.reshape(B, NP, 128, FWP)).astype(bfloat16)


def _make_sones():
    """[128, 5*20] bf16 stationaries; S_d[(r,c), m] = 1/32 iff m == r*5+d."""
    S = np.zeros((128, NOFF * M), dtype=np.float32)
    for d in range(NOFF):
        for r in range(NR):
            S[r * C:(r + 1) * C, d * M + r * NOFF + d] = 1.0 / C
    return S.astype(bfloat16)


def _run(c1_full, warph_full, trace=False, **kw):
    from concourse.bass_utils import run_bass_kernel_spmd

    nc = _build()
    c1t = _prep_c1(c1_full)
    sones = _make_sones()
    in_maps = [{"c1t": c1t[i], "wpt": warph_full[i], "sones": sones}
               for i in range(B)]
    return run_bass_kernel_spmd(nc, in_maps, list(range(B)), trace=trace, **kw)


def kernel(c1, warp, search_range):
    assert int(search_range) == SR, f"kernel hardcodes search_range={SR}"
    c1 = np.ascontiguousarray(np.asarray(c1, dtype=np.float32))
    warp = np.ascontiguousarray(np.asarray(warp, dtype=np.float32))
    assert c1.shape == (B, H, W, C) and warp.shape == (B, H, W, C)
    warph = _prep_warph(warp)
    r = _run(c1, warph, trace=False)
    out = np.empty((B, H, W, OUTC), dtype=np.float32)
    out[..., :C] = c1
    for i in range(B):
        cost = np.asarray(r.results[i]["out"]).astype(np.float32)
        # [NP, s, p=(g, q<=31), w]; valid q<20 encode (r, d) = (q//5, q%5)
        cost = cost.reshape(NP, NS, NSUB, 32, W)[:, :, :, :M, :]
        cost = cost.reshape(NP, NS, NSUB, NR, NOFF, W)
        cost = cost.transpose(0, 1, 2, 3, 5, 4)     # P s g r w d
        out[i, ..., C:] = cost.reshape(H, W, NOFF)
    return out
